# revision 6
# baseline (speedup 1.0000x reference)
"""Trainium2 Bass kernel for nn_MultiHeadAttention (B=2, T=2048, D=768, H=12).

Sharding over 8 NeuronCores: core id c = (b<<2) | (qh<<1) | hh where
  b  = batch (2)
  qh = query-half of the sequence (2 x 1024 rows)
  hh = head-half (2 x 6 heads = 384 features)

Each core computes, for its (b, qh, hh):
  - Q^T/K^T in feature-major layout via fp32r matmuls (inputs host-transposed)
  - V in token-major layout, augmented with a ones column per head
  - per head: S^T[k,q] = K_h Q_h^T (PE), Pt = exp(s*S^T) in fp16 (ACT,
    no max-subtraction needed: |s*S| <= ~6), then outT_aug[65,q] = [V|1]^T Pt
    giving both the unnormalized attention output (feature-major) and the
    softmax row sums in one accumulation group.
  - normalization of the small outT via a PE transpose dance (per-q scale),
    reciprocal row sums r, and a broadcast R = 1/(12*s) tile used to
    accumulate mean-over-heads attention probabilities in fp16.
  - Wo partial projection (feature-sliced), fp32r.

Host side: transposes inputs/outputs, sums the two hh partials, adds biases
bo; attn_mean = (acc_hh0 + acc_hh1)^T.
"""

import sys

sys.path.insert(0, "/opt/trn_rl_repo")

import numpy as np

import concourse.bass as bass
import concourse.tile as tile
from concourse import bacc, mybir
from concourse.masks import make_identity

B, T, D, H, HS = 2, 2048, 768, 12, 64
QS = 1024          # query rows per core
NH = 6             # heads per core
F = NH * HS        # 384 features per core
NCORES = 8
SCALE = 1.0 / float(np.sqrt(HS))

f32 = mybir.dt.float32
f32r = mybir.dt.float32r
f16 = mybir.dt.float16
Act = mybir.ActivationFunctionType

_CACHE = {}


def _build_program():
    nc = bacc.Bacc("TRN2", target_bir_lowering=False, debug=False,
                   num_devices=NCORES)

    # ---- DRAM parameters (per core) ----
    qT = nc.declare_dram_parameter("qT", [D, QS], f32r, isOutput=False)
    kT = nc.declare_dram_parameter("kT", [D, T], f32r, isOutput=False)
    vT = nc.declare_dram_parameter("vT", [D, T], f32r, isOutput=False)
    WqT = nc.declare_dram_parameter("WqT", [D, F], f32r, isOutput=False)
    WkT = nc.declare_dram_parameter("WkT", [D, F], f32r, isOutput=False)
    WvT = nc.declare_dram_parameter("WvT", [D, F], f32r, isOutput=False)
    WoT = nc.declare_dram_parameter("WoT", [F, D], f32r, isOutput=False)
    bq2 = nc.declare_dram_parameter("bq2", [128, 3], f32, isOutput=False)
    bk2 = nc.declare_dram_parameter("bk2", [128, 3], f32, isOutput=False)
    bvb = nc.declare_dram_parameter("bvb", [128, F], f32, isOutput=False)
    accT = nc.declare_dram_parameter("accT", [T, QS], f16, isOutput=True)
    outT = nc.declare_dram_parameter("outT", [D, QS], f32, isOutput=True)

    DT = D // 128   # 6 input-feature tiles
    FT = F // 128   # 3 our-feature tiles
    KT_T = T // 128  # 16 key-token tiles
    QT_Q = QS // 128  # 8 query-token tiles

    from contextlib import ExitStack
    stack = ExitStack()
    with tile.TileContext(nc) as tc, stack:
        consts = stack.enter_context(tc.tile_pool(name="consts", bufs=1))
        projp = stack.enter_context(tc.tile_pool(name="projp", bufs=1))
        psA = stack.enter_context(tc.tile_pool(name="psA", bufs=2, space="PSUM"))

        # ---- constants ----
        ident = consts.tile([128, 128], f32)
        make_identity(nc, ident[:])
        ones_f = consts.tile([1, 128], f32)
        nc.vector.memset(ones_f[:], 1.0)
        ones_col = consts.tile([1, 128], f32r)
        nc.vector.tensor_copy(ones_col[:], ones_f[:])

        wo = [consts.tile([128, D], f32r, tag=f"wo{i}", name=f"wo{i}")
              for i in range(FT)]
        for i in range(FT):
            nc.sync.dma_start(out=wo[i][:], in_=WoT.ap()[i * 128:(i + 1) * 128, :])
        bq2s = consts.tile([128, 3], f32)
        bk2s = consts.tile([128, 3], f32)
        bvbs = consts.tile([128, F], f32)
        nc.sync.dma_start(out=bq2s[:], in_=bq2.ap())
        nc.sync.dma_start(out=bk2s[:], in_=bk2.ap())
        nc.sync.dma_start(out=bvbs[:], in_=bvb.ap())

        # ---- persistent projection outputs ----
        QTt = [projp.tile([128, QS], f32r, tag=f"QT{i}", name=f"QTt{i}")
               for i in range(FT)]
        KTt = [projp.tile([128, T], f32r, tag=f"KT{i}", name=f"KTt{i}")
               for i in range(FT)]
        Vaug = [projp.tile([128, NH, HS + 1], f16, tag=f"Va{i}", name=f"Va{i}")
                for i in range(KT_T)]
        AOT = [projp.tile([128, QS], f32r, tag=f"AOT{i}", name=f"AOT{i}")
               for i in range(FT)]

        # ---- phase 1: load x + weights, projections ----
        with tc.tile_pool(name="xp", bufs=1) as xp, \
                tc.tile_pool(name="psVp", bufs=2, space="PSUM") as psVp:
            wq = [xp.tile([128, F], f32r, tag=f"wq{i}", name=f"wq{i}")
                  for i in range(DT)]
            wk = [xp.tile([128, F], f32r, tag=f"wk{i}", name=f"wk{i}")
                  for i in range(DT)]
            wv = [xp.tile([128, F], f32r, tag=f"wv{i}", name=f"wv{i}")
                  for i in range(DT)]
            for i in range(DT):
                nc.sync.dma_start(out=wq[i][:], in_=WqT.ap()[i * 128:(i + 1) * 128, :])
                nc.sync.dma_start(out=wk[i][:], in_=WkT.ap()[i * 128:(i + 1) * 128, :])
                nc.sync.dma_start(out=wv[i][:], in_=WvT.ap()[i * 128:(i + 1) * 128, :])
            qx = [xp.tile([128, QS], f32r, tag=f"qx{i}", name=f"qx{i}")
                  for i in range(DT)]
            for i in range(DT):
                nc.sync.dma_start(out=qx[i][:], in_=qT.ap()[i * 128:(i + 1) * 128, :])

            # Q^T projection (feature-major)
            for ot in range(FT):
                ps = psA.tile([128, QS], f32, tag="psA", name="psq")
                for dt_i in range(DT):
                    for nn_ in range(QS // 512):
                        nc.tensor.matmul(
                            ps[:, nn_ * 512:(nn_ + 1) * 512],
                            lhsT=wq[dt_i][:, ot * 128:(ot + 1) * 128],
                            rhs=qx[dt_i][:, nn_ * 512:(nn_ + 1) * 512],
                            start=(dt_i == 0), stop=(dt_i == DT - 1))
                nc.scalar.activation(QTt[ot][:], ps[:], Act.Identity,
                                     bias=bq2s[:, ot:ot + 1])

            # K^T projection, halved over key tokens
            for half in range(2):
                kxh = [xp.tile([128, 1024], f32r, tag=f"kxh{i}", name=f"kxh{i}")
                       for i in range(DT)]
                for i in range(DT):
                    nc.sync.dma_start(
                        out=kxh[i][:],
                        in_=kT.ap()[i * 128:(i + 1) * 128,
                                    half * 1024:(half + 1) * 1024])
                for ot in range(FT):
                    ps = psA.tile([128, 1024], f32, tag="psA", name="psk")
                    for dt_i in range(DT):
                        for nn_ in range(2):
                            nc.tensor.matmul(
                                ps[:, nn_ * 512:(nn_ + 1) * 512],
                                lhsT=wk[dt_i][:, ot * 128:(ot + 1) * 128],
                                rhs=kxh[dt_i][:, nn_ * 512:(nn_ + 1) * 512],
                                start=(dt_i == 0), stop=(dt_i == DT - 1))
                    nc.scalar.activation(
                        KTt[ot][:, half * 1024:(half + 1) * 1024], ps[:],
                        Act.Identity, bias=bk2s[:, ot:ot + 1])

            # V natural projection + bias + ones col, halved over tokens
            for half in range(2):
                vxh = [xp.tile([128, 1024], f32r, tag=f"vxh{i}", name=f"vxh{i}")
                       for i in range(DT)]
                for i in range(DT):
                    nc.sync.dma_start(
                        out=vxh[i][:],
                        in_=vT.ap()[i * 128:(i + 1) * 128,
                                    half * 1024:(half + 1) * 1024])
                for tl in range(8):
                    tt = half * 8 + tl
                    ps = psVp.tile([128, F], f32, tag="psV", name="psv")
                    for dt_i in range(DT):
                        nc.tensor.matmul(
                            ps[:], lhsT=vxh[dt_i][:, tl * 128:(tl + 1) * 128],
                            rhs=wv[dt_i][:],
                            start=(dt_i == 0), stop=(dt_i == DT - 1))
                    va = Vaug[tt]
                    nc.vector.memset(va[:, :, HS:HS + 1], 1.0)
                    nc.vector.tensor_add(
                        va[:, :, 0:HS],
                        ps[:].rearrange("p (h s) -> p h s", h=NH),
                        bvbs[:].rearrange("p (h s) -> p h s", h=NH))

        # ---- phase 2: attention per head ----
        ptp = stack.enter_context(tc.tile_pool(name="ptp", bufs=2))
        accp = stack.enter_context(tc.tile_pool(name="accp", bufs=1))
        smallp = stack.enter_context(tc.tile_pool(name="smallp", bufs=2))
        psAug = stack.enter_context(tc.tile_pool(name="psAug", bufs=1, space="PSUM"))
        psD = stack.enter_context(tc.tile_pool(name="psD", bufs=2, space="PSUM"))

        acc = accp.tile([128, KT_T, QS], f16)
        for h in range(NH):
            th, hb = h // 2, (h % 2) * 64
            Pt = ptp.tile([128, KT_T, QS], f16, tag="Pt", name=f"Pt{h}")
            aug = psAug.tile([HS + 1, QS], f32, tag="aug", name=f"aug{h}")
            for kt in range(KT_T):
                st = psA.tile([128, QS], f32, tag="psA", name=f"st{h}_{kt}")
                for nn_ in range(QS // 512):
                    nc.tensor.matmul(
                        st[:, nn_ * 512:(nn_ + 1) * 512],
                        lhsT=KTt[th][hb:hb + 64, kt * 128:(kt + 1) * 128],
                        rhs=QTt[th][hb:hb + 64, nn_ * 512:(nn_ + 1) * 512],
                        start=True, stop=True)
                nc.scalar.activation(Pt[:, kt, :], st[:], Act.Exp, scale=SCALE)
                for nn_ in range(QS // 512):
                    nc.tensor.matmul(
                        aug[:, nn_ * 512:(nn_ + 1) * 512],
                        lhsT=Vaug[kt][:, h, :],
                        rhs=Pt[:, kt, nn_ * 512:(nn_ + 1) * 512],
                        start=(kt == 0), stop=(kt == KT_T - 1),
                        skip_group_check=True)
            # normalization dance
            augs = smallp.tile([HS + 1, QS], f32, tag="augs", name=f"augs{h}")
            nc.scalar.copy(augs[:], aug[:])
            r12all = smallp.tile([128, 8], f32, tag="r12", name=f"r12_{h}")
            for qt in range(QT_Q):
                tp = psD.tile([128, 128], f32, tag="psD", name=f"tp{h}_{qt}")
                nc.tensor.transpose(tp[:, 0:HS + 1],
                                    augs[:, qt * 128:(qt + 1) * 128],
                                    ident[0:HS + 1, 0:HS + 1])
                r = smallp.tile([128, 1], f32, tag="rr", name=f"r{h}_{qt}")
                nc.vector.reciprocal(r[:], tp[:, HS:HS + 1])
                nc.vector.tensor_scalar_mul(r12all[:, qt:qt + 1], r[:],
                                            1.0 / 12.0)
                non = smallp.tile([128, HS], f32, tag="non", name=f"non{h}_{qt}")
                nc.vector.tensor_scalar(non[:], tp[:, 0:HS], r[:], None,
                                        mybir.AluOpType.mult)
                tp2 = psD.tile([128, 128], f32, tag="psD", name=f"tp2{h}_{qt}")
                nc.tensor.transpose(tp2[0:HS, :], non[:], ident[:, :])
                nc.scalar.activation(
                    AOT[th][hb:hb + 64, qt * 128:(qt + 1) * 128],
                    tp2[0:HS, :], Act.Identity)
            # R broadcast tile: R[k, q] = 1/(12*s[q]) for all k
            tpr = psD.tile([128, 128], f32, tag="psD", name=f"tpr{h}")
            nc.tensor.transpose(tpr[0:8, :], r12all[:], ident[:, :])
            rr_s = smallp.tile([8, 128], f32r, tag="rrs", name=f"rrs{h}")
            nc.scalar.copy(rr_s[:], tpr[0:8, :])
            rrow = smallp.tile([1, QS], f32r, tag="rrow", name=f"rrow{h}")
            nc.sync.dma_start(
                out=rrow[:].rearrange("p (a b) -> p a b", a=8),
                in_=rr_s[:].rearrange("a (p b) -> a p b", p=1))
            rb = psAug.tile([128, QS], f32, tag="aug", name=f"rb{h}")
            for nn_ in range(QS // 512):
                nc.tensor.matmul(rb[:, nn_ * 512:(nn_ + 1) * 512],
                                 lhsT=ones_col[:],
                                 rhs=rrow[:, nn_ * 512:(nn_ + 1) * 512],
                                 start=True, stop=True)
            Rt = smallp.tile([128, QS], f16, tag="Rt", name=f"Rt{h}")
            nc.scalar.copy(Rt[:], rb[:])
            # accumulate mean over heads: acc += Pt * R
            for kt in range(KT_T):
                if h == 0:
                    nc.vector.tensor_mul(acc[:, kt, :], Pt[:, kt, :], Rt[:])
                else:
                    nc.vector.tensor_mul(Pt[:, kt, :], Pt[:, kt, :], Rt[:])
                    nc.vector.tensor_add(acc[:, kt, :], acc[:, kt, :],
                                         Pt[:, kt, :])

        # ---- phase 3: Wo projection + stores ----
        for ot in range(DT):
            ps = psA.tile([128, QS], f32, tag="psA", name=f"pso{ot}")
            for ct in range(FT):
                for nn_ in range(QS // 512):
                    nc.tensor.matmul(
                        ps[:, nn_ * 512:(nn_ + 1) * 512],
                        lhsT=wo[ct][:, ot * 128:(ot + 1) * 128],
                        rhs=AOT[ct][:, nn_ * 512:(nn_ + 1) * 512],
                        start=(ct == 0), stop=(ct == FT - 1))
            oo = smallp.tile([128, QS], f32, tag="oo", name=f"oo{ot}")
            nc.scalar.copy(oo[:], ps[:])
            nc.sync.dma_start(out=outT.ap()[ot * 128:(ot + 1) * 128, :],
                              in_=oo[:])
        for kt in range(KT_T):
            nc.sync.dma_start(out=accT.ap()[kt * 128:(kt + 1) * 128, :],
                              in_=acc[:, kt, :])

    nc.compile()
    return nc


def _get_runner():
    if "runner" in _CACHE:
        return _CACHE["runner"]

    import jax
    from jax.sharding import Mesh, PartitionSpec
    from jax.experimental.shard_map import shard_map
    from concourse import bass2jax
    from concourse.bass2jax import (_bass_exec_p, install_neuronx_cc_hook,
                                    partition_id_tensor)

    nc = _build_program()
    install_neuronx_cc_hook()

    pid_name = nc.partition_id_tensor.name if nc.partition_id_tensor else None
    in_names, out_names, out_avals, zero_shapes = [], [], [], []
    for alloc in nc.m.functions[0].allocations:
        if not isinstance(alloc, mybir.MemoryLocationSet):
            continue
        name = alloc.memorylocations[0].name
        if alloc.kind == "ExternalInput":
            if name != pid_name:
                in_names.append(name)
        elif alloc.kind == "ExternalOutput":
            out_names.append(name)
            shape = tuple(alloc.tensor_shape)
            dtype = mybir.dt.np(alloc.dtype)
            out_avals.append(jax.core.ShapedArray(shape, dtype))
            zero_shapes.append((shape, dtype))
    n_params = len(in_names)
    all_in_names = in_names + out_names
    if pid_name is not None:
        all_in_names = all_in_names + [pid_name]

    def _body(*args):
        operands = list(args)
        if pid_name is not None:
            operands.append(partition_id_tensor())
        outs = _bass_exec_p.bind(
            *operands,
            out_avals=tuple(out_avals),
            in_names=tuple(all_in_names),
            out_names=tuple(out_names),
            lowering_input_output_aliases=(),
            sim_require_finite=True,
            sim_require_nnan=True,
            nc=nc,
        )
        return tuple(outs)

    devices = jax.devices()[:NCORES]
    mesh = Mesh(np.asarray(devices), ("core",))
    n_outs = len(out_names)
    sharded = jax.jit(
        shard_map(_body, mesh=mesh,
                  in_specs=(PartitionSpec("core"),) * (n_params + n_outs),
                  out_specs=(PartitionSpec("core"),) * n_outs,
                  check_rep=False),
        keep_unused=True)

    runner = {
        "nc": nc, "sharded": sharded, "in_names": in_names,
        "out_names": out_names, "zero_shapes": zero_shapes,
        "out_avals": out_avals, "jax": jax,
    }
    _CACHE["runner"] = runner
    return runner


def _prep_core_inputs(query, key, value, Wq, bq, Wk, bk, Wv, bv, Wo, bo):
    """Returns list of 8 dicts of per-core input arrays."""
    C = np.ascontiguousarray
    f = np.float32
    maps = []
    for cid in range(NCORES):
        b, qh, hh = cid >> 2, (cid >> 1) & 1, cid & 1
        fsl = slice(hh * F, (hh + 1) * F)
        qsl = slice(qh * QS, (qh + 1) * QS)
        maps.append({
            "qT": C(query[b, qsl, :].T.astype(f)),
            "kT": C(key[b].T.astype(f)),
            "vT": C(value[b].T.astype(f)),
            "WqT": C(Wq[fsl, :].T.astype(f)),
            "WkT": C(Wk[fsl, :].T.astype(f)),
            "WvT": C(Wv[fsl, :].T.astype(f)),
            "WoT": C(Wo[:, fsl].T.astype(f)),
            "bq2": C(bq[fsl].astype(f).reshape(3, 128).T),
            "bk2": C(bk[fsl].astype(f).reshape(3, 128).T),
            "bvb": C(np.broadcast_to(bv[fsl].astype(f), (128, F))),
        })
    return maps


def _device_args(maps):
    r = _get_runner()
    jax = r["jax"]
    concat = []
    for name in r["in_names"]:
        concat.append(np.concatenate([m[name] for m in maps], axis=0))
    for shape, dtype in r["zero_shapes"]:
        concat.append(np.zeros((NCORES * shape[0],) + shape[1:], dtype))
    return [jax.device_put(a) for a in concat]


def _exec(args):
    r = _get_runner()
    outs = r["sharded"](*args)
    res = []
    for c in range(NCORES):
        d = {}
        for i, name in enumerate(r["out_names"]):
            shape = r["out_avals"][i].shape
            d[name] = np.asarray(outs[i]).reshape((NCORES,) + shape)[c]
        res.append(d)
    return res


def _assemble(res, bo):
    output = np.empty((B, T, D), np.float32)
    attn = np.empty((B, T, T), np.float32)
    for b in range(B):
        for qh in range(2):
            c0 = (b << 2) | (qh << 1)
            c1 = c0 | 1
            ot = res[c0]["outT"] + res[c1]["outT"]
            output[b, qh * QS:(qh + 1) * QS, :] = ot.T + np.asarray(bo, np.float32)
            at = (res[c0]["accT"].astype(np.float32)
                  + res[c1]["accT"].astype(np.float32))
            attn[b, qh * QS:(qh + 1) * QS, :] = at.T
    return output, attn


def kernel(query, key, value, Wq, bq, Wk, bk, Wv, bv, Wo, bo):
    maps = _prep_core_inputs(query, key, value, Wq, bq, Wk, bk, Wv, bv, Wo, bo)
    args = _device_args(maps)
    res = _exec(args)
    return _assemble(res, bo)


def timed_run(inputs, iters=20, warmup=3):
    """Returns (mean_seconds_per_iter, result). Amortizes dispatch overhead
    by submitting all iterations asynchronously before blocking."""
    import time
    r = _get_runner()
    maps = _prep_core_inputs(**inputs)
    args = _device_args(maps)
    for _ in range(warmup):
        outs = r["sharded"](*args)
    for o in outs:
        o.block_until_ready()
    t0 = time.perf_counter()
    for _ in range(iters):
        outs = r["sharded"](*args)
    for o in outs:
        o.block_until_ready()
    t1 = time.perf_counter()
    return (t1 - t0) / iters
